# revision 23
# baseline (speedup 1.0000x reference)
"""Causal self-attention (B=4, T=2048, C=1024, H=16) on 8 TRN2 NeuronCores.

Sharding: tensor-parallel over heads. Each core owns 2 of the 16 heads and
produces a partial (B*T, C) output; the host sums the 8 partials.

v2 design notes (vs the earlier baseline at ~766us):
  - The TRN2 PE clock is HAM-gated: it only reaches 2.4 GHz under sustained
    matmul activity and falls to 1.2 GHz after idle windows.  The baseline's
    attention phase ran almost entirely cold.  v2 weaves stage-A qkv chunks
    of batch b+1, sampled stats of pair p+1, and the projection of batch b-1
    into each pair's score strips so the PE instruction stream never starves.
  - The separate full stats (row-max) pass is replaced by a sampled max:
    for query tile qt, 128 strided columns of the causal prefix are scored
    and max-reduced.  The max may be under-estimated by a few sigma, so P is
    kept in bf16 (range e^+-88) instead of fp16; softmax is shift-invariant
    so any bounded shift is exact.  Query tile 0 uses m=0 (sigma there is
    small enough that exp stays in range).
  - x is shipped once ([C, B*T] fp16); the log(t)^alpha/sqrt(D) position
    scale is folded into the PSUM->SBUF copy of q as a DVE multiply with a
    per-row scale tile, so the old second pre-scaled copy of x is gone.
  - exp runs on ACT at [128, 1024] grain (two 512-col score tiles per PSUM
    tile) to amortize per-instruction overhead; ACT does nothing else in
    steady state.  Mask adds / reductions / normalize run on DVE, constant
    generation and odds and ends on Pool, and half the projection output is
    DMA'd to HBM as f32 directly from PSUM to keep ACT/DVE off the critical
    path.
"""

import sys

if "/opt/trn_rl_repo" not in sys.path:
    sys.path.insert(0, "/opt/trn_rl_repo")

import math

import numpy as np

# ---------------------------------------------------------------- constants
B, T, C, H, D = 4, 2048, 1024, 16, 64
ALPHA = 2.0
NCORES = 8
HPC = H // NCORES          # heads per core = 2
NP = B * HPC               # (batch, head) pairs per core = 8
BT = B * T                 # 8192 rows
KC = C // 128              # 8 contraction tiles for the qkv projection
CH = 512                   # stage-A row chunk / score strip width
NCH = BT // CH             # 16 chunks
QTPB = T // 128            # 16 query tiles per batch
SPB = T // CH              # 4 query strips per batch
NEG = -1.0e9

_F16 = np.float16


def _build_nc():
    import concourse.mybir as mybir
    from concourse import bacc
    from concourse.masks import make_identity
    from concourse.tile import TileContext

    f16 = mybir.dt.float16
    bf16 = mybir.dt.bfloat16
    f32 = mybir.dt.float32
    AX = mybir.AxisListType.X

    nc = bacc.Bacc()

    xT = nc.dram_tensor("xT", [C, BT], f16, kind="ExternalInput")
    sv = nc.dram_tensor("sv", [D, T], f16, kind="ExternalInput")
    wq = nc.dram_tensor("wq", [C, HPC * D], f16, kind="ExternalInput")
    wk = nc.dram_tensor("wk", [C, HPC * D], f16, kind="ExternalInput")
    wv = nc.dram_tensor("wv", [C, HPC * D], f16, kind="ExternalInput")
    wp = nc.dram_tensor("wp", [HPC * D, C], f16, kind="ExternalInput")
    out = nc.dram_tensor("out", [BT, C], f16, kind="ExternalOutput")

    with TileContext(nc) as tc:
        with (
            tc.tile_pool(name="persist", bufs=1) as pp,
            tc.tile_pool(name="xin", bufs=2) as xp,
            tc.tile_pool(name="ptile", bufs=4) as ptp,
            tc.tile_pool(name="otile", bufs=2) as otp,
            tc.tile_pool(name="small", bufs=2) as sp,
            tc.tile_pool(name="tiny", bufs=4) as tp,
            tc.tile_pool(name="psS", bufs=2, space="PSUM") as psS,
            tc.tile_pool(name="psO", bufs=4, space="PSUM") as psO,
        ):
            # ---- persistent tiles
            qsT = pp.tile([65, NP, T], f16, tag="qsT")        # q'^T + bias row
            kaT = pp.tile([65, NP, T], f16, tag="kaT")        # k^T + ones row
            vA = pp.tile([128, NP, QTPB, 65], bf16, tag="vA")  # v + ones col
            yT = pp.tile([128, BT], f16, tag="yT")            # y^T, both heads
            wqs = pp.tile([128, KC, 128], f16, tag="wqs")
            wks = pp.tile([128, KC, 128], f16, tag="wks")
            wvs = pp.tile([128, KC, 128], f16, tag="wvs")
            wps = pp.tile([128, C], f16, tag="wps")
            stile = pp.tile([D, T], f16, tag="stile")         # pos scale rows
            ident = pp.tile([128, 128], f32, tag="ident")
            tri01 = pp.tile([128, 128], bf16, tag="tri01")    # [k,q]: 1 if k<=q

            # ---- init constants
            nc.sync.dma_start(out=wqs, in_=wq[:, :].rearrange("(kt p) n -> p kt n", p=128))
            nc.sync.dma_start(out=wks, in_=wk[:, :].rearrange("(kt p) n -> p kt n", p=128))
            nc.sync.dma_start(out=wvs, in_=wv[:, :].rearrange("(kt p) n -> p kt n", p=128))
            nc.sync.dma_start(out=stile, in_=sv[:, :])
            make_identity(nc, ident)
            idx = pp.tile([128, 128], mybir.dt.int32, tag="idx")
            nc.gpsimd.iota(idx, pattern=[[1, 128]], base=0, channel_multiplier=-1)
            nc.vector.tensor_scalar(
                out=tri01, in0=idx, scalar1=0, scalar2=None,
                op0=mybir.AluOpType.is_ge)
            nc.gpsimd.memset(vA[:, :, :, 64:65], 1.0)
            nc.gpsimd.memset(kaT[64:65, :, :], 1.0)

            # ---- stage A: qkv projection for one 512-row chunk
            def emit_chunk(n):
                b, loc = n // SPB, (n % SPB) * CH
                xt = xp.tile([128, KC, CH], f16, tag="xt")
                nc.sync.dma_start(
                    out=xt,
                    in_=xT[:, n * CH:(n + 1) * CH].rearrange(
                        "(kt p) r -> p kt r", p=128))
                psq = psO.tile([128, CH], f32, tag="out")
                for kt in range(KC):
                    nc.tensor.matmul(psq, wqs[:, kt, :], xt[:, kt, :],
                                     start=(kt == 0), stop=(kt == KC - 1))
                psk = psO.tile([128, CH], f32, tag="out")
                for kt in range(KC):
                    nc.tensor.matmul(psk, wks[:, kt, :], xt[:, kt, :],
                                     start=(kt == 0), stop=(kt == KC - 1))
                for h in range(HPC):
                    pair = b * HPC + h
                    # q: fused position-scale multiply (scale along rows)
                    nc.vector.tensor_mul(
                        qsT[0:64, pair, loc:loc + CH],
                        psq[h * 64:(h + 1) * 64, :],
                        stile[:, loc:loc + CH])
                    nc.scalar.copy(
                        kaT[0:64, pair, loc:loc + CH],
                        psk[h * 64:(h + 1) * 64, :])
                psv = psO.tile([128, CH], f32, tag="out")
                for sub in range(CH // 128):
                    for kt in range(KC):
                        nc.tensor.matmul(
                            psv[:, sub * 128:(sub + 1) * 128],
                            xt[:, kt, sub * 128:(sub + 1) * 128],
                            wvs[:, kt, :],
                            start=(kt == 0), stop=(kt == KC - 1))
                psv3 = psv[:, :].rearrange("p (s c) -> p s c", s=CH // 128)
                kt0 = (n % SPB) * (CH // 128)
                for h in range(HPC):
                    pair = b * HPC + h
                    nc.scalar.copy(
                        vA[:, pair, kt0:kt0 + CH // 128, 0:64],
                        psv3[:, :, h * 64:(h + 1) * 64])

            # ---- sampled row-max stats for one query tile (qt >= 1)
            m_alls = {}

            def get_m_all(pair):
                if pair not in m_alls:
                    m_alls[pair] = sp.tile(
                        [128, QTPB], f32, tag="mall", name="m_all")
                    # qt = 0 rows use m = 0 (pos scale is small there)
                    nc.gpsimd.memset(m_alls[pair][:, 0:1], 0.0)
                return m_alls[pair]

            def emit_stats_qt(pair, qt):
                m_all = get_m_all(pair)
                pool = qt * 128
                ks = kaT[0:64, pair, 0:pool].rearrange(
                    "p (n s) -> p n s", s=qt)[:, :, 0:1]
                ps = psO.tile([128, CH], f32, tag="out")
                nc.tensor.matmul(
                    ps[:, 0:128],
                    qsT[0:64, pair, qt * 128:(qt + 1) * 128],
                    ks, start=True, stop=True)
                nc.vector.reduce_max(
                    m_all[:, qt:qt + 1], ps[:, 0:128], axis=AX)

            def emit_mchain(pair):
                m_all = m_alls.pop(pair)
                pmt = psO.tile([16, 128], f32, tag="out")
                nc.tensor.transpose(pmt, m_all, ident)
                # bias row = -(m_hat + 8): the extra -8 keeps denominators
                # comfortably below reciprocal_approx_fast's ~1e38 limit
                mrow = tp.tile([16, 128], f16, tag="mrow")
                nc.scalar.activation(
                    mrow, pmt, mybir.ActivationFunctionType.Copy,
                    bias=-8.0, scale=-1.0)
                nc.sync.dma_start(out=qsT[64:65, pair, :], in_=mrow)

            # ---- one score strip: S^T tiles -> exp -> PV accumulation
            def emit_st_strip(pair, qs, fill):
                """fill: list of zero-arg callables; one is popped and run
                after each S^T/PV tile pair to keep other engines fed."""
                y_ps = psO.tile([65, CH], f32, tag="out")
                kts = 4 * (qs + 1)
                for kth in range(kts // 2):
                    ps = psS.tile([128, 2 * CH], f32, tag="sc")
                    offs = []
                    for half in range(2):
                        kt = 2 * kth + half
                        off = max(0, kt * 128 - qs * CH)
                        offs.append(off)
                        nc.tensor.matmul(
                            ps[:, half * CH + off:(half + 1) * CH],
                            kaT[0:65, pair, kt * 128:(kt + 1) * 128],
                            qsT[0:65, pair, qs * CH + off:(qs + 1) * CH],
                            start=True, stop=True)
                    pt = ptp.tile([128, 2 * CH], bf16, tag="pt")
                    nc.scalar.activation(
                        pt[:, offs[0]:2 * CH], ps[:, offs[0]:2 * CH],
                        mybir.ActivationFunctionType.Exp)
                    for half in range(2):
                        kt = 2 * kth + half
                        off = offs[half]
                        if kt >= 4 * qs:
                            # causal mask: zero the k>q block (post-exp, all-
                            # SBUF bf16 so DVE runs in high-throughput mode)
                            nc.vector.tensor_mul(
                                pt[:, half * CH + off:half * CH + off + 128],
                                pt[:, half * CH + off:half * CH + off + 128],
                                tri01)
                    for half in range(2):
                        kt = 2 * kth + half
                        off = offs[half]
                        nc.tensor.matmul(
                            y_ps[:, off:CH],
                            vA[:, pair, kt, :],
                            pt[:, half * CH + off:(half + 1) * CH],
                            start=(kt == 0), stop=(kt == kts - 1))
                    if fill:
                        fill.pop(0)()
                return y_ps

            # ---- per-strip normalize: yT = y / denom
            def emit_normalize(pair, qs, y_ps):
                b, h = pair // HPC, pair % HPC
                drow = tp.tile([1, CH], f32, tag="drow")
                nc.vector.tensor_copy(drow, y_ps[64:65, :])
                rec = tp.tile([1, CH], f32, tag="rec")
                nc.vector.reciprocal_approx_fast(rec, drow)
                dbc = sp.tile([64, CH], f32, tag="dbc")
                nc.gpsimd.partition_broadcast(dbc, rec, channels=64)
                nc.vector.tensor_mul(
                    yT[h * 64:(h + 1) * 64,
                       b * T + qs * CH:b * T + (qs + 1) * CH],
                    y_ps[0:64, :], dbc)

            # ---- projection of one row tile (both output halves)
            def emit_proj_rt(b, rt):
                r0 = b * T + rt * 128
                for nt in range(2):
                    po = psO.tile([128, CH], f32, tag="out")
                    nc.tensor.matmul(
                        po, yT[:, r0:r0 + 128],
                        wps[:, nt * CH:(nt + 1) * CH],
                        start=True, stop=True)
                    ot = otp.tile([128, CH], f16, tag="ot")
                    if (rt + nt) % 2 == 0:
                        nc.scalar.copy(ot, po)
                    else:
                        nc.vector.tensor_copy(ot, po)
                    nc.sync.dma_start(
                        out=out[r0:r0 + 128, nt * CH:(nt + 1) * CH], in_=ot)

            # ---------------------------------------------------- schedule
            for n in range(SPB):           # batch 0 stage A
                emit_chunk(n)
            nc.sync.dma_start(out=wps, in_=wp[:, :])
            for qt in range(1, QTPB):      # pair 0 stats
                emit_stats_qt(0, qt)
            get_m_all(0)
            emit_mchain(0)

            # chunk weaving: batch b+1's four chunks split across the two
            # pairs of batch b -- (pair 2b: strips 1,3 -> chunks 0,1) and
            # (pair 2b+1: strips 0,1 -> chunks 2,3).  stats for the next
            # pair only ever need chunks already emitted.
            chunk_slot = {(0, 1): 0, (0, 3): 1, (1, 0): 2, (1, 1): 3}
            for p in range(NP):
                b = p // HPC
                stats_qts = {0: range(1, 6), 1: range(6, 11), 2: range(11, 16)}
                for qs in range(SPB):
                    fill = []
                    if p + 1 < NP:
                        for qt in stats_qts.get(qs, ()):
                            fill.append(
                                lambda pair=p + 1, q=qt: emit_stats_qt(pair, q))
                        if qs == 3:
                            fill.append(lambda pair=p + 1: emit_mchain(pair))
                    ck = chunk_slot.get((p % 2, qs))
                    if ck is not None and b + 1 < B:
                        fill.append(lambda n=(b + 1) * SPB + ck: emit_chunk(n))
                    if p % 2 == 1 and qs > 0:
                        for rt in range(4 * (qs - 1), 4 * qs):
                            fill.append(lambda bb=b, r=rt: emit_proj_rt(bb, r))
                    y_ps = emit_st_strip(p, qs, fill)
                    emit_normalize(p, qs, y_ps)
                    for f in fill:
                        f()
                if p % 2 == 1:
                    for rt in range(12, 16):
                        emit_proj_rt(b, rt)
    nc.compile()
    return nc


_NC_CACHE = None
TRACE = False           # set by test harness for profiling runs
LAST_RESULT = None      # BassKernelResults of the last run (when TRACE)


def kernel(x, w_attn, w_proj):
    global _NC_CACHE, LAST_RESULT
    from concourse.bass_utils import run_bass_kernel_spmd

    if _NC_CACHE is None:
        _NC_CACHE = _build_nc()
    nc = _NC_CACHE

    x2 = np.asarray(x, dtype=np.float32).reshape(BT, C)
    pos = np.arange(1, T + 1, dtype=np.float64)
    svv = (np.log(pos) ** ALPHA / math.sqrt(D)).astype(np.float32)
    sv_tile = np.broadcast_to(svv[None, :], (D, T)).astype(_F16)
    xT = np.ascontiguousarray(x2.T).astype(_F16)
    wa = np.asarray(w_attn, dtype=np.float32)
    wpj = np.asarray(w_proj, dtype=np.float32)

    in_maps = []
    for c in range(NCORES):
        h0 = c * HPC
        cols = np.r_[h0 * D:(h0 + HPC) * D]
        in_maps.append({
            "xT": xT,
            "sv": sv_tile,
            "wq": np.ascontiguousarray(wa[:, cols]).astype(_F16),
            "wk": np.ascontiguousarray(wa[:, C + cols]).astype(_F16),
            "wv": np.ascontiguousarray(wa[:, 2 * C + cols]).astype(_F16),
            "wp": np.ascontiguousarray(wpj[cols, :]).astype(_F16),
        })

    res = run_bass_kernel_spmd(
        nc, in_maps, core_ids=list(range(NCORES)), trace=TRACE)
    LAST_RESULT = res
    total = np.zeros((BT, C), dtype=np.float32)
    for r in res.results:
        total += r["out"].astype(np.float32)
    return total.reshape(B, T, C)


# revision 32
# speedup vs baseline: 1.0002x; 1.0002x over previous
"""Causal self-attention (B=4, T=2048, C=1024, H=16) on 8 TRN2 NeuronCores.

Sharding: tensor-parallel over heads. Each core owns 2 of the 16 heads and
produces a partial (B*T, C) output; the host sums the 8 partials.

v2 design notes (vs the earlier baseline at ~766us):
  - The TRN2 PE clock is HAM-gated: it only reaches 2.4 GHz under sustained
    matmul activity and falls to 1.2 GHz after idle windows.  The baseline's
    attention phase ran almost entirely cold.  v2 weaves stage-A qkv chunks
    of batch b+1, sampled stats of pair p+1, and the projection of batch b-1
    into each pair's score strips so the PE instruction stream never starves.
  - The separate full stats (row-max) pass is replaced by a sampled max:
    for query tile qt, 128 strided columns of the causal prefix are scored
    and max-reduced.  The max may be under-estimated by a few sigma, so P is
    kept in bf16 (range e^+-88) instead of fp16; softmax is shift-invariant
    so any bounded shift is exact.  Query tile 0 uses m=0 (sigma there is
    small enough that exp stays in range).
  - x is shipped once ([C, B*T] fp16); the log(t)^alpha/sqrt(D) position
    scale is folded into the PSUM->SBUF copy of q as a DVE multiply with a
    per-row scale tile, so the old second pre-scaled copy of x is gone.
  - exp runs on ACT at [128, 1024] grain (two 512-col score tiles per PSUM
    tile) to amortize per-instruction overhead; ACT does nothing else in
    steady state.  Mask adds / reductions / normalize run on DVE, constant
    generation and odds and ends on Pool, and half the projection output is
    DMA'd to HBM as f32 directly from PSUM to keep ACT/DVE off the critical
    path.
"""

import sys

if "/opt/trn_rl_repo" not in sys.path:
    sys.path.insert(0, "/opt/trn_rl_repo")

import math

import numpy as np

# ---------------------------------------------------------------- constants
B, T, C, H, D = 4, 2048, 1024, 16, 64
ALPHA = 2.0
NCORES = 8
HPC = H // NCORES          # heads per core = 2
NP = B * HPC               # (batch, head) pairs per core = 8
BT = B * T                 # 8192 rows
KC = C // 128              # 8 contraction tiles for the qkv projection
CH = 512                   # stage-A row chunk / score strip width
NCH = BT // CH             # 16 chunks
QTPB = T // 128            # 16 query tiles per batch
SPB = T // CH              # 4 query strips per batch
NEG = -1.0e9

_F16 = np.float16


def _build_nc():
    import concourse.mybir as mybir
    from concourse import bacc
    from concourse.masks import make_identity
    from concourse.tile import TileContext

    f16 = mybir.dt.float16
    bf16 = mybir.dt.bfloat16
    f32 = mybir.dt.float32
    AX = mybir.AxisListType.X

    nc = bacc.Bacc()

    xT = nc.dram_tensor("xT", [C, BT], f16, kind="ExternalInput")
    sv = nc.dram_tensor("sv", [D, T], f16, kind="ExternalInput")
    wq = nc.dram_tensor("wq", [C, HPC * D], f16, kind="ExternalInput")
    wk = nc.dram_tensor("wk", [C, HPC * D], f16, kind="ExternalInput")
    wv = nc.dram_tensor("wv", [C, HPC * D], f16, kind="ExternalInput")
    wp = nc.dram_tensor("wp", [HPC * D, C], f16, kind="ExternalInput")
    out = nc.dram_tensor("out", [BT, C], f16, kind="ExternalOutput")

    with TileContext(nc) as tc:
        with (
            tc.tile_pool(name="persist", bufs=1) as pp,
            tc.tile_pool(name="xin", bufs=5) as xp,
            tc.tile_pool(name="ptile", bufs=4) as ptp,
            tc.tile_pool(name="otile", bufs=2) as otp,
            tc.tile_pool(name="small", bufs=2) as sp,
            tc.tile_pool(name="tiny", bufs=4) as tp,
            tc.tile_pool(name="psS", bufs=2, space="PSUM") as psS,
            tc.tile_pool(name="psO", bufs=4, space="PSUM") as psO,
        ):
            # ---- persistent tiles
            qsT = pp.tile([65, NP, T], f16, tag="qsT")        # q'^T + bias row
            kaT = pp.tile([65, NP, T], f16, tag="kaT")        # k^T + ones row
            vA = pp.tile([128, NP, QTPB, 65], bf16, tag="vA")  # v + ones col
            yT = pp.tile([128, BT], f16, tag="yT")            # y^T, both heads
            wqs = pp.tile([128, KC, 128], f16, tag="wqs")
            wks = pp.tile([128, KC, 128], f16, tag="wks")
            wvs = pp.tile([128, KC, 128], f16, tag="wvs")
            wps = pp.tile([128, C], f16, tag="wps")
            stile = pp.tile([D, T], f16, tag="stile")         # pos scale rows
            ident = pp.tile([128, 128], f32, tag="ident")
            tri01 = pp.tile([128, 128], bf16, tag="tri01")    # [k,q]: 1 if k<=q

            # ---- init constants
            nc.sync.dma_start(out=wqs, in_=wq[:, :].rearrange("(kt p) n -> p kt n", p=128))
            nc.sync.dma_start(out=wks, in_=wk[:, :].rearrange("(kt p) n -> p kt n", p=128))
            nc.sync.dma_start(out=wvs, in_=wv[:, :].rearrange("(kt p) n -> p kt n", p=128))
            nc.sync.dma_start(out=stile, in_=sv[:, :])
            make_identity(nc, ident)
            idx = pp.tile([128, 128], mybir.dt.int32, tag="idx")
            nc.gpsimd.iota(idx, pattern=[[1, 128]], base=0, channel_multiplier=-1)
            nc.vector.tensor_scalar(
                out=tri01, in0=idx, scalar1=0, scalar2=None,
                op0=mybir.AluOpType.is_ge)
            nc.gpsimd.memset(vA[:, :, :, 64:65], 1.0)
            nc.gpsimd.memset(kaT[64:65, :, :], 1.0)

            # ---- stage A: qkv projection for one 512-row chunk
            def fetch_x(n):
                xt = xp.tile([128, KC, CH], f16, tag="xt")
                nc.sync.dma_start(
                    out=xt,
                    in_=xT[:, n * CH:(n + 1) * CH].rearrange(
                        "(kt p) r -> p kt r", p=128))
                return xt

            def emit_chunk(n, xt=None):
                b, loc = n // SPB, (n % SPB) * CH
                if xt is None:
                    xt = fetch_x(n)
                psq = psO.tile([128, CH], f32, tag="out")
                for kt in range(KC):
                    nc.tensor.matmul(psq, wqs[:, kt, :], xt[:, kt, :],
                                     start=(kt == 0), stop=(kt == KC - 1))
                psk = psO.tile([128, CH], f32, tag="out")
                for kt in range(KC):
                    nc.tensor.matmul(psk, wks[:, kt, :], xt[:, kt, :],
                                     start=(kt == 0), stop=(kt == KC - 1))
                for h in range(HPC):
                    pair = b * HPC + h
                    # q: fused position-scale multiply (scale along rows)
                    nc.vector.tensor_mul(
                        qsT[0:64, pair, loc:loc + CH],
                        psq[h * 64:(h + 1) * 64, :],
                        stile[:, loc:loc + CH])
                    nc.scalar.copy(
                        kaT[0:64, pair, loc:loc + CH],
                        psk[h * 64:(h + 1) * 64, :])
                psv = psO.tile([128, CH], f32, tag="out")
                for sub in range(CH // 128):
                    for kt in range(KC):
                        nc.tensor.matmul(
                            psv[:, sub * 128:(sub + 1) * 128],
                            xt[:, kt, sub * 128:(sub + 1) * 128],
                            wvs[:, kt, :],
                            start=(kt == 0), stop=(kt == KC - 1))
                psv3 = psv[:, :].rearrange("p (s c) -> p s c", s=CH // 128)
                kt0 = (n % SPB) * (CH // 128)
                for h in range(HPC):
                    pair = b * HPC + h
                    nc.scalar.copy(
                        vA[:, pair, kt0:kt0 + CH // 128, 0:64],
                        psv3[:, :, h * 64:(h + 1) * 64])

            # ---- sampled row-max stats for one query tile (qt >= 1)
            m_alls = {}

            def get_m_all(pair):
                if pair not in m_alls:
                    m_alls[pair] = sp.tile(
                        [128, QTPB], f32, tag="mall", name="m_all")
                    # qt = 0 rows use m = 0 (pos scale is small there)
                    nc.gpsimd.memset(m_alls[pair][:, 0:1], 0.0)
                return m_alls[pair]

            def emit_stats_pair(pair, qt0, nqt):
                """Sampled-max stats for query tiles qt0..qt0+nqt-1 with a
                single batched reduce (nqt <= 2, consecutive)."""
                m_all = get_m_all(pair)
                for j in range(nqt):
                    qt = qt0 + j
                    ks = kaT[0:64, pair, 0:qt * 128].rearrange(
                        "p (n s) -> p n s", s=qt)[:, :, 0:1]
                    ps = psO.tile([128, CH], f32, tag="out")
                    nc.tensor.matmul(
                        ps[:, 0:128],
                        qsT[0:64, pair, qt * 128:(qt + 1) * 128],
                        ks, start=True, stop=True)
                    nc.vector.reduce_max(
                        m_all[:, qt:qt + 1], ps[:, 0:128], axis=AX)

            def emit_mchain(pair):
                m_all = m_alls.pop(pair)
                pmt = psO.tile([16, 128], f32, tag="out")
                nc.tensor.transpose(pmt, m_all, ident)
                # bias row = -(m_hat + 8): the extra -8 keeps denominators
                # comfortably below reciprocal_approx_fast's ~1e38 limit
                mrow = tp.tile([16, 128], f16, tag="mrow")
                nc.scalar.activation(
                    mrow, pmt, mybir.ActivationFunctionType.Copy,
                    bias=-8.0, scale=-1.0)
                nc.sync.dma_start(out=qsT[64:65, pair, :], in_=mrow)

            # ---- one score strip: S^T tiles -> exp -> PV accumulation
            def emit_st_strip(pair, qs, fill):
                """fill: list of zero-arg callables; one is popped and run
                after each S^T/PV tile pair to keep other engines fed."""
                y_ps = psO.tile([65, CH], f32, tag="out")
                kts = 4 * (qs + 1)
                for kth in range(kts // 2):
                    ps = psS.tile([128, 2 * CH], f32, tag="sc")
                    offs = []
                    for half in range(2):
                        kt = 2 * kth + half
                        off = max(0, kt * 128 - qs * CH)
                        offs.append(off)
                        nc.tensor.matmul(
                            ps[:, half * CH + off:(half + 1) * CH],
                            kaT[0:65, pair, kt * 128:(kt + 1) * 128],
                            qsT[0:65, pair, qs * CH + off:(qs + 1) * CH],
                            start=True, stop=True)
                    pt = ptp.tile([128, 2 * CH], bf16, tag="pt")
                    nc.scalar.activation(
                        pt[:, offs[0]:2 * CH], ps[:, offs[0]:2 * CH],
                        mybir.ActivationFunctionType.Exp)
                    for half in range(2):
                        kt = 2 * kth + half
                        off = offs[half]
                        if kt >= 4 * qs:
                            # causal mask: zero the k>q block (post-exp, all-
                            # SBUF bf16 so DVE runs in high-throughput mode)
                            nc.vector.tensor_mul(
                                pt[:, half * CH + off:half * CH + off + 128],
                                pt[:, half * CH + off:half * CH + off + 128],
                                tri01)
                    for half in range(2):
                        kt = 2 * kth + half
                        off = offs[half]
                        nc.tensor.matmul(
                            y_ps[:, off:CH],
                            vA[:, pair, kt, :],
                            pt[:, half * CH + off:(half + 1) * CH],
                            start=(kt == 0), stop=(kt == kts - 1))
                    if fill:
                        fill.pop(0)()
                return y_ps

            # ---- per-strip normalize: yT = y / denom
            def emit_normalize(pair, qs, y_ps):
                b, h = pair // HPC, pair % HPC
                drow = tp.tile([1, CH], f32, tag="drow")
                nc.vector.tensor_copy(drow, y_ps[64:65, :])
                rec = tp.tile([1, CH], f32, tag="rec")
                nc.vector.reciprocal_approx_fast(rec, drow)
                dbc = sp.tile([64, CH], f32, tag="dbc")
                nc.gpsimd.partition_broadcast(dbc, rec, channels=64)
                nc.vector.tensor_mul(
                    yT[h * 64:(h + 1) * 64,
                       b * T + qs * CH:b * T + (qs + 1) * CH],
                    y_ps[0:64, :], dbc)

            # ---- projection of one row tile (both output halves)
            def emit_proj_rt(b, rt):
                r0 = b * T + rt * 128
                for nt in range(2):
                    po = psO.tile([128, CH], f32, tag="out")
                    nc.tensor.matmul(
                        po, yT[:, r0:r0 + 128],
                        wps[:, nt * CH:(nt + 1) * CH],
                        start=True, stop=True)
                    ot = otp.tile([128, CH], f16, tag="ot")
                    if (rt + nt) % 4 == 0:
                        nc.scalar.copy(ot, po)
                    else:
                        nc.vector.tensor_copy(ot, po)
                    nc.sync.dma_start(
                        out=out[r0:r0 + 128, nt * CH:(nt + 1) * CH], in_=ot)

            # ---------------------------------------------------- schedule
            xts = [fetch_x(n) for n in range(SPB)]   # prefetch batch 0
            for n in range(SPB):           # batch 0 stage A
                emit_chunk(n, xts[n])
            nc.sync.dma_start(out=wps, in_=wp[:, :])
            for qt0 in range(1, QTPB, 2):  # pair 0 stats
                emit_stats_pair(0, qt0, min(2, QTPB - qt0))
            get_m_all(0)
            emit_mchain(0)

            # chunk weaving: batch b+1's four chunks split across the two
            # pairs of batch b -- (pair 2b: strips 1,3 -> chunks 0,1) and
            # (pair 2b+1: strips 0,1 -> chunks 2,3).  stats for the next
            # pair only ever need chunks already emitted.
            chunk_slot = {(0, 1): 0, (0, 3): 1, (1, 0): 2, (1, 1): 3}
            for p in range(NP):
                b = p // HPC
                # stats for pair p+1 must not be emitted before the stage-A
                # chunk that writes the q rows it reads: on odd pairs, chunks
                # 2/3 of the next batch land in strips 0/1, so query tiles
                # 8-11 (chunk 2) wait for strip 1 and 12-15 (chunk 3) for
                # strip 2.
                if p % 2 == 0:
                    stats_qts = {0: [(1, 2), (3, 2), (5, 2)],
                                 1: [(7, 2), (9, 2), (11, 2)],
                                 2: [(13, 2), (15, 1)]}
                else:
                    stats_qts = {0: [(1, 2), (3, 2), (5, 2), (7, 1)],
                                 1: [(8, 2), (10, 2)],
                                 2: [(12, 2), (14, 2)]}
                for qs in range(SPB):
                    fill = []
                    if p + 1 < NP:
                        for qt0, nq in stats_qts.get(qs, ()):
                            fill.append(
                                lambda pair=p + 1, q=qt0, n=nq:
                                emit_stats_pair(pair, q, n))
                        if qs == 3:
                            fill.append(lambda pair=p + 1: emit_mchain(pair))
                    ck = chunk_slot.get((p % 2, qs))
                    if ck is not None and b + 1 < B:
                        fill.append(lambda n=(b + 1) * SPB + ck: emit_chunk(n))
                    if p % 2 == 1 and qs > 0:
                        for rt in range(4 * (qs - 1), 4 * qs):
                            fill.append(lambda bb=b, r=rt: emit_proj_rt(bb, r))
                    y_ps = emit_st_strip(p, qs, fill)
                    emit_normalize(p, qs, y_ps)
                    for f in fill:
                        f()
                if p % 2 == 1:
                    for rt in range(12, 16):
                        emit_proj_rt(b, rt)
    nc.compile()
    return nc


_NC_CACHE = None
TRACE = False           # set by test harness for profiling runs
LAST_RESULT = None      # BassKernelResults of the last run (when TRACE)


def kernel(x, w_attn, w_proj):
    global _NC_CACHE, LAST_RESULT
    from concourse.bass_utils import run_bass_kernel_spmd

    if _NC_CACHE is None:
        _NC_CACHE = _build_nc()
    nc = _NC_CACHE

    x2 = np.asarray(x, dtype=np.float32).reshape(BT, C)
    pos = np.arange(1, T + 1, dtype=np.float64)
    svv = (np.log(pos) ** ALPHA / math.sqrt(D)).astype(np.float32)
    sv_tile = np.broadcast_to(svv[None, :], (D, T)).astype(_F16)
    xT = np.ascontiguousarray(x2.T).astype(_F16)
    wa = np.asarray(w_attn, dtype=np.float32)
    wpj = np.asarray(w_proj, dtype=np.float32)

    in_maps = []
    for c in range(NCORES):
        h0 = c * HPC
        cols = np.r_[h0 * D:(h0 + HPC) * D]
        in_maps.append({
            "xT": xT,
            "sv": sv_tile,
            "wq": np.ascontiguousarray(wa[:, cols]).astype(_F16),
            "wk": np.ascontiguousarray(wa[:, C + cols]).astype(_F16),
            "wv": np.ascontiguousarray(wa[:, 2 * C + cols]).astype(_F16),
            "wp": np.ascontiguousarray(wpj[cols, :]).astype(_F16),
        })

    res = run_bass_kernel_spmd(
        nc, in_maps, core_ids=list(range(NCORES)), trace=TRACE)
    LAST_RESULT = res
    total = np.zeros((BT, C), dtype=np.float32)
    for r in res.results:
        total += r["out"].astype(np.float32)
    return total.reshape(B, T, C)


# revision 33
# speedup vs baseline: 1.0320x; 1.0318x over previous
"""Causal self-attention (B=4, T=2048, C=1024, H=16) on 8 TRN2 NeuronCores.

Sharding: tensor-parallel over heads. Each core owns 2 of the 16 heads and
produces a partial (B*T, C) output; the host sums the 8 partials.

v2 design notes (vs the earlier baseline at ~766us):
  - The TRN2 PE clock is HAM-gated: it only reaches 2.4 GHz under sustained
    matmul activity and falls to 1.2 GHz after idle windows.  The baseline's
    attention phase ran almost entirely cold.  v2 weaves stage-A qkv chunks
    of batch b+1, sampled stats of pair p+1, and the projection of batch b-1
    into each pair's score strips so the PE instruction stream never starves.
  - The separate full stats (row-max) pass is replaced by a sampled max:
    for query tile qt, 128 strided columns of the causal prefix are scored
    and max-reduced.  The max may be under-estimated by a few sigma, so P is
    kept in bf16 (range e^+-88) instead of fp16; softmax is shift-invariant
    so any bounded shift is exact.  Query tile 0 uses m=0 (sigma there is
    small enough that exp stays in range).
  - x is shipped once ([C, B*T] fp16); the log(t)^alpha/sqrt(D) position
    scale is folded into the PSUM->SBUF copy of q as a DVE multiply with a
    per-row scale tile, so the old second pre-scaled copy of x is gone.
  - exp runs on ACT at [128, 1024] grain (two 512-col score tiles per PSUM
    tile) to amortize per-instruction overhead; ACT does nothing else in
    steady state.  Mask adds / reductions / normalize run on DVE, constant
    generation and odds and ends on Pool, and half the projection output is
    DMA'd to HBM as f32 directly from PSUM to keep ACT/DVE off the critical
    path.
"""

import sys

if "/opt/trn_rl_repo" not in sys.path:
    sys.path.insert(0, "/opt/trn_rl_repo")

import math

import numpy as np

# ---------------------------------------------------------------- constants
B, T, C, H, D = 4, 2048, 1024, 16, 64
ALPHA = 2.0
NCORES = 8
HPC = H // NCORES          # heads per core = 2
NP = B * HPC               # (batch, head) pairs per core = 8
BT = B * T                 # 8192 rows
KC = C // 128              # 8 contraction tiles for the qkv projection
CH = 512                   # stage-A row chunk / score strip width
NCH = BT // CH             # 16 chunks
QTPB = T // 128            # 16 query tiles per batch
SPB = T // CH              # 4 query strips per batch
NEG = -1.0e9

_F16 = np.float16


def _build_nc():
    import concourse.mybir as mybir
    from concourse import bacc
    from concourse.masks import make_identity
    from concourse.tile import TileContext

    f16 = mybir.dt.float16
    bf16 = mybir.dt.bfloat16
    f32 = mybir.dt.float32
    AX = mybir.AxisListType.X

    nc = bacc.Bacc()

    xT = nc.dram_tensor("xT", [C, BT], f16, kind="ExternalInput")
    sv = nc.dram_tensor("sv", [D, T], f16, kind="ExternalInput")
    wq = nc.dram_tensor("wq", [C, HPC * D], f16, kind="ExternalInput")
    wk = nc.dram_tensor("wk", [C, HPC * D], f16, kind="ExternalInput")
    wv = nc.dram_tensor("wv", [C, HPC * D], f16, kind="ExternalInput")
    wp = nc.dram_tensor("wp", [HPC * D, C], f16, kind="ExternalInput")
    out = nc.dram_tensor("out", [BT, C], f16, kind="ExternalOutput")

    with TileContext(nc) as tc:
        with (
            tc.tile_pool(name="persist", bufs=1) as pp,
            tc.tile_pool(name="xin", bufs=5) as xp,
            tc.tile_pool(name="ptile", bufs=4) as ptp,
            tc.tile_pool(name="otile", bufs=2) as otp,
            tc.tile_pool(name="small", bufs=2) as sp,
            tc.tile_pool(name="tiny", bufs=4) as tp,
            tc.tile_pool(name="psS", bufs=2, space="PSUM") as psS,
            tc.tile_pool(name="psO", bufs=4, space="PSUM") as psO,
        ):
            # ---- persistent tiles
            qsT = pp.tile([65, NP, T], f16, tag="qsT")        # q'^T + bias row
            kaT = pp.tile([65, NP, T], f16, tag="kaT")        # k^T + ones row
            vA = pp.tile([128, NP, QTPB, 65], bf16, tag="vA")  # v + ones col
            yT = pp.tile([128, BT], f16, tag="yT")            # y^T, both heads
            wqs = pp.tile([128, KC, 128], f16, tag="wqs")
            wks = pp.tile([128, KC, 128], f16, tag="wks")
            wvs = pp.tile([128, KC, 128], f16, tag="wvs")
            wps = pp.tile([128, C], f16, tag="wps")
            stile = pp.tile([D, T], f16, tag="stile")         # pos scale rows
            ident = pp.tile([128, 128], f32, tag="ident")
            tri01 = pp.tile([128, 128], bf16, tag="tri01")    # [k,q]: 1 if k<=q

            # ---- init constants
            nc.sync.dma_start(out=wqs, in_=wq[:, :].rearrange("(kt p) n -> p kt n", p=128))
            nc.sync.dma_start(out=wks, in_=wk[:, :].rearrange("(kt p) n -> p kt n", p=128))
            nc.sync.dma_start(out=wvs, in_=wv[:, :].rearrange("(kt p) n -> p kt n", p=128))
            nc.sync.dma_start(out=stile, in_=sv[:, :])
            make_identity(nc, ident)
            idx = pp.tile([128, 128], mybir.dt.int32, tag="idx")
            nc.gpsimd.iota(idx, pattern=[[1, 128]], base=0, channel_multiplier=-1)
            nc.vector.tensor_scalar(
                out=tri01, in0=idx, scalar1=0, scalar2=None,
                op0=mybir.AluOpType.is_ge)
            nc.gpsimd.memset(vA[:, :, :, 64:65], 1.0)
            nc.gpsimd.memset(kaT[64:65, :, :], 1.0)

            # ---- stage A: qkv projection for one 512-row chunk
            def fetch_x(n):
                xt = xp.tile([128, KC, CH], f16, tag="xt")
                nc.sync.dma_start(
                    out=xt,
                    in_=xT[:, n * CH:(n + 1) * CH].rearrange(
                        "(kt p) r -> p kt r", p=128))
                return xt

            def emit_chunk(n, xt=None):
                b, loc = n // SPB, (n % SPB) * CH
                if xt is None:
                    xt = fetch_x(n)
                psq = psO.tile([128, CH], f32, tag="out")
                for kt in range(KC):
                    nc.tensor.matmul(psq, wqs[:, kt, :], xt[:, kt, :],
                                     start=(kt == 0), stop=(kt == KC - 1))
                psk = psO.tile([128, CH], f32, tag="out")
                for kt in range(KC):
                    nc.tensor.matmul(psk, wks[:, kt, :], xt[:, kt, :],
                                     start=(kt == 0), stop=(kt == KC - 1))
                for h in range(HPC):
                    pair = b * HPC + h
                    # q: fused position-scale multiply (scale along rows)
                    nc.vector.tensor_mul(
                        qsT[0:64, pair, loc:loc + CH],
                        psq[h * 64:(h + 1) * 64, :],
                        stile[:, loc:loc + CH])
                    nc.scalar.copy(
                        kaT[0:64, pair, loc:loc + CH],
                        psk[h * 64:(h + 1) * 64, :])
                psv = psO.tile([128, CH], f32, tag="out")
                for sub in range(CH // 128):
                    for kt in range(KC):
                        nc.tensor.matmul(
                            psv[:, sub * 128:(sub + 1) * 128],
                            xt[:, kt, sub * 128:(sub + 1) * 128],
                            wvs[:, kt, :],
                            start=(kt == 0), stop=(kt == KC - 1))
                psv3 = psv[:, :].rearrange("p (s c) -> p s c", s=CH // 128)
                kt0 = (n % SPB) * (CH // 128)
                for h in range(HPC):
                    pair = b * HPC + h
                    nc.scalar.copy(
                        vA[:, pair, kt0:kt0 + CH // 128, 0:64],
                        psv3[:, :, h * 64:(h + 1) * 64])

            # ---- sampled row-max stats for one query tile (qt >= 1)
            m_alls = {}

            def get_m_all(pair):
                if pair not in m_alls:
                    m_alls[pair] = sp.tile(
                        [128, QTPB], f32, tag="mall", name="m_all")
                    # qt = 0 rows use m = 0 (pos scale is small there)
                    nc.gpsimd.memset(m_alls[pair][:, 0:1], 0.0)
                return m_alls[pair]

            def emit_stats_pair(pair, qt0, nqt):
                """Sampled-max stats for query tiles qt0..qt0+nqt-1 with a
                single batched reduce (nqt <= 2, consecutive)."""
                m_all = get_m_all(pair)
                for j in range(nqt):
                    qt = qt0 + j
                    ks = kaT[0:64, pair, 0:qt * 128].rearrange(
                        "p (n s) -> p n s", s=qt)[:, :, 0:1]
                    ps = psO.tile([128, CH], f32, tag="out")
                    nc.tensor.matmul(
                        ps[:, 0:128],
                        qsT[0:64, pair, qt * 128:(qt + 1) * 128],
                        ks, start=True, stop=True)
                    nc.vector.reduce_max(
                        m_all[:, qt:qt + 1], ps[:, 0:128], axis=AX)

            def emit_mchain(pair):
                m_all = m_alls.pop(pair)
                pmt = psO.tile([16, 128], f32, tag="out")
                nc.tensor.transpose(pmt, m_all, ident)
                # bias row = -(m_hat + 8): the extra -8 keeps denominators
                # comfortably below reciprocal_approx_fast's ~1e38 limit
                mrow = tp.tile([16, 128], f16, tag="mrow")
                nc.scalar.activation(
                    mrow, pmt, mybir.ActivationFunctionType.Copy,
                    bias=-8.0, scale=-1.0)
                nc.sync.dma_start(out=qsT[64:65, pair, :], in_=mrow)

            # ---- one score strip: S^T tiles -> exp -> PV accumulation
            def emit_st_strip(pair, qs, fill):
                """fill: list of zero-arg callables; one is popped and run
                after each S^T/PV tile pair to keep other engines fed."""
                y_ps = psO.tile([65, CH], f32, tag="out")
                kts = 4 * (qs + 1)

                def emit_pv(pt, offs, kth):
                    for half in range(2):
                        kt = 2 * kth + half
                        off = offs[half]
                        nc.tensor.matmul(
                            y_ps[:, off:CH],
                            vA[:, pair, kt, :],
                            pt[:, half * CH + off:(half + 1) * CH],
                            start=(kt == 0), stop=(kt == kts - 1))

                pend = None      # (pt, offs, kth) — PV lags one tile
                for kth in range(kts // 2):
                    ps = psS.tile([128, 2 * CH], f32, tag="sc")
                    offs = []
                    for half in range(2):
                        kt = 2 * kth + half
                        off = max(0, kt * 128 - qs * CH)
                        offs.append(off)
                        nc.tensor.matmul(
                            ps[:, half * CH + off:(half + 1) * CH],
                            kaT[0:65, pair, kt * 128:(kt + 1) * 128],
                            qsT[0:65, pair, qs * CH + off:(qs + 1) * CH],
                            start=True, stop=True)
                    pt = ptp.tile([128, 2 * CH], bf16, tag="pt")
                    nc.scalar.activation(
                        pt[:, offs[0]:2 * CH], ps[:, offs[0]:2 * CH],
                        mybir.ActivationFunctionType.Exp)
                    for half in range(2):
                        kt = 2 * kth + half
                        off = offs[half]
                        if kt >= 4 * qs:
                            # causal mask: zero the k>q block (post-exp, all-
                            # SBUF bf16 so DVE runs in high-throughput mode)
                            nc.vector.tensor_mul(
                                pt[:, half * CH + off:half * CH + off + 128],
                                pt[:, half * CH + off:half * CH + off + 128],
                                tri01)
                    if pend is not None:
                        emit_pv(*pend)
                    pend = (pt, offs, kth)
                    if fill:
                        fill.pop(0)()
                emit_pv(*pend)
                return y_ps

            # ---- per-strip normalize: yT = y / denom
            def emit_normalize(pair, qs, y_ps):
                b, h = pair // HPC, pair % HPC
                drow = tp.tile([1, CH], f32, tag="drow")
                nc.vector.tensor_copy(drow, y_ps[64:65, :])
                rec = tp.tile([1, CH], f32, tag="rec")
                nc.vector.reciprocal_approx_fast(rec, drow)
                dbc = sp.tile([64, CH], f32, tag="dbc")
                nc.gpsimd.partition_broadcast(dbc, rec, channels=64)
                nc.vector.tensor_mul(
                    yT[h * 64:(h + 1) * 64,
                       b * T + qs * CH:b * T + (qs + 1) * CH],
                    y_ps[0:64, :], dbc)

            # ---- projection of one row tile (both output halves)
            def emit_proj_rt(b, rt):
                r0 = b * T + rt * 128
                for nt in range(2):
                    po = psO.tile([128, CH], f32, tag="out")
                    nc.tensor.matmul(
                        po, yT[:, r0:r0 + 128],
                        wps[:, nt * CH:(nt + 1) * CH],
                        start=True, stop=True)
                    ot = otp.tile([128, CH], f16, tag="ot")
                    if (rt + nt) % 4 == 0:
                        nc.scalar.copy(ot, po)
                    else:
                        nc.vector.tensor_copy(ot, po)
                    nc.sync.dma_start(
                        out=out[r0:r0 + 128, nt * CH:(nt + 1) * CH], in_=ot)

            # ---------------------------------------------------- schedule
            xts = [fetch_x(n) for n in range(SPB)]   # prefetch batch 0
            for n in range(SPB):           # batch 0 stage A
                emit_chunk(n, xts[n])
            nc.sync.dma_start(out=wps, in_=wp[:, :])
            for qt0 in range(1, QTPB, 2):  # pair 0 stats
                emit_stats_pair(0, qt0, min(2, QTPB - qt0))
            get_m_all(0)
            emit_mchain(0)

            # chunk weaving: batch b+1's four chunks split across the two
            # pairs of batch b -- (pair 2b: strips 1,3 -> chunks 0,1) and
            # (pair 2b+1: strips 0,1 -> chunks 2,3).  stats for the next
            # pair only ever need chunks already emitted.
            chunk_slot = {(0, 1): 0, (0, 3): 1, (1, 0): 2, (1, 1): 3}
            for p in range(NP):
                b = p // HPC
                # stats for pair p+1 must not be emitted before the stage-A
                # chunk that writes the q rows it reads: on odd pairs, chunks
                # 2/3 of the next batch land in strips 0/1, so query tiles
                # 8-11 (chunk 2) wait for strip 1 and 12-15 (chunk 3) for
                # strip 2.
                if p % 2 == 0:
                    stats_qts = {0: [(1, 2), (3, 2), (5, 2)],
                                 1: [(7, 2), (9, 2), (11, 2)],
                                 2: [(13, 2), (15, 1)]}
                else:
                    stats_qts = {0: [(1, 2), (3, 2), (5, 2), (7, 1)],
                                 1: [(8, 2), (10, 2)],
                                 2: [(12, 2), (14, 2)]}
                for qs in range(SPB):
                    fill = []
                    if p + 1 < NP:
                        for qt0, nq in stats_qts.get(qs, ()):
                            fill.append(
                                lambda pair=p + 1, q=qt0, n=nq:
                                emit_stats_pair(pair, q, n))
                        if qs == 3:
                            fill.append(lambda pair=p + 1: emit_mchain(pair))
                    ck = chunk_slot.get((p % 2, qs))
                    if ck is not None and b + 1 < B:
                        fill.append(lambda n=(b + 1) * SPB + ck: emit_chunk(n))
                    if p % 2 == 1 and qs > 0:
                        for rt in range(4 * (qs - 1), 4 * qs):
                            fill.append(lambda bb=b, r=rt: emit_proj_rt(bb, r))
                    y_ps = emit_st_strip(p, qs, fill)
                    emit_normalize(p, qs, y_ps)
                    for f in fill:
                        f()
                if p % 2 == 1:
                    for rt in range(12, 16):
                        emit_proj_rt(b, rt)
    nc.compile()
    return nc


_NC_CACHE = None
TRACE = False           # set by test harness for profiling runs
LAST_RESULT = None      # BassKernelResults of the last run (when TRACE)


def kernel(x, w_attn, w_proj):
    global _NC_CACHE, LAST_RESULT
    from concourse.bass_utils import run_bass_kernel_spmd

    if _NC_CACHE is None:
        _NC_CACHE = _build_nc()
    nc = _NC_CACHE

    x2 = np.asarray(x, dtype=np.float32).reshape(BT, C)
    pos = np.arange(1, T + 1, dtype=np.float64)
    svv = (np.log(pos) ** ALPHA / math.sqrt(D)).astype(np.float32)
    sv_tile = np.broadcast_to(svv[None, :], (D, T)).astype(_F16)
    xT = np.ascontiguousarray(x2.T).astype(_F16)
    wa = np.asarray(w_attn, dtype=np.float32)
    wpj = np.asarray(w_proj, dtype=np.float32)

    in_maps = []
    for c in range(NCORES):
        h0 = c * HPC
        cols = np.r_[h0 * D:(h0 + HPC) * D]
        in_maps.append({
            "xT": xT,
            "sv": sv_tile,
            "wq": np.ascontiguousarray(wa[:, cols]).astype(_F16),
            "wk": np.ascontiguousarray(wa[:, C + cols]).astype(_F16),
            "wv": np.ascontiguousarray(wa[:, 2 * C + cols]).astype(_F16),
            "wp": np.ascontiguousarray(wpj[cols, :]).astype(_F16),
        })

    res = run_bass_kernel_spmd(
        nc, in_maps, core_ids=list(range(NCORES)), trace=TRACE)
    LAST_RESULT = res
    total = np.zeros((BT, C), dtype=np.float32)
    for r in res.results:
        total += r["out"].astype(np.float32)
    return total.reshape(B, T, C)
